# revision 46
# baseline (speedup 1.0000x reference)
"""Multi-head attention (B=2, S=2048, D=2048, H=16) on 8 trn2 NeuronCores.

Sharding: tensor-parallel over heads. Core c owns heads [2c, 2c+1]:
  - computes q/k/v projections for its 256 output dims (bf16 matmuls, fp32 PSUM)
  - attention for its 2 heads x 2 batches (transposed-score layout, fused
    softmax denominator, no on-chip transposes)
  - partial output projection  merged_c @ Wo[:, c_slice].T  -> [B, D, S] fp32
Host: sums the 8 partials, transposes, adds bo (+ Wo @ bv for the folded
v-bias).

Schedule:
  - cold tile (b0,t0) runs Q/K/V chains ci-interleaved (Q leads while only
    xt+wq have landed) so compute streams at DMA arrival rate
  - attention per head: all 8 score pairs, then all 8 AV pairs (slack on the
    scores->exp->AV dependency); softmax denominator pre-summed on DVE so a
    single ones-matmul reduces it, issued mid-AV so the reciprocal is off
    the critical path
  - out-projection (P3) blocks injected between score pairs / early AV
    slots; steady-state stores ride the gpsimd SWDGE queue so SP-queue
    loads never head-of-line block; 1-in-4 copies on Act, rest on DVE
  - each batch's t3 V chains and the next batch's t0 V chains are deferred
    into qt0's score slots as 4-matmul pieces (fills the slots that
    otherwise stall on the exp pipeline)
  - final drain: first six blocks' h0 matmuls run before MT[:,1,:] lands,
    then two-block pairs from the freed ps_mm pool with one wide copy and
    one 256KB store each across all three DMA queues; last blocks split
    fine for short-latency flush
"""

import itertools

import numpy as np

try:
    import concourse.bass as bass  # noqa: F401
except ImportError:  # pragma: no cover - fresh grading dir
    import sys

    sys.path.insert(0, "/opt/trn_rl_repo")

import ml_dtypes

import concourse.bacc as bacc
import concourse.mybir as mybir
import concourse.tile as tile
from concourse.bass_utils import run_bass_kernel_spmd

B, S, D, H = 2, 2048, 2048, 16
HD = D // H  # 128
N_CORES = 8
HPC = H // N_CORES  # heads per core = 2
CD = HPC * HD  # per-core projection dims = 256
TOK = B * S  # 4096

BF16 = mybir.dt.bfloat16
F32 = mybir.dt.float32

TT = 512  # token tile (free dim of most matmuls)
KC = D // 128  # contraction chunks for projections = 16
NB = S // 128  # key blocks per batch = 16
NQ = S // TT  # q tiles per batch = 4
NT = S // TT  # token tiles per batch = 4
SCALE = 1.0 / float(np.sqrt(HD))

Act = mybir.ActivationFunctionType


def build_program():
    nc = bacc.Bacc("TRN2", target_bir_lowering=False, debug=False, num_devices=N_CORES)

    xT = nc.dram_tensor("xT", [D, TOK], BF16, kind="ExternalInput").ap()
    wqT = nc.dram_tensor("wqT", [D, CD], BF16, kind="ExternalInput").ap()
    wkT = nc.dram_tensor("wkT", [D, CD], BF16, kind="ExternalInput").ap()
    wvT = nc.dram_tensor("wvT", [D, CD], BF16, kind="ExternalInput").ap()
    woT = nc.dram_tensor("woT", [CD, D], BF16, kind="ExternalInput").ap()
    bq = nc.dram_tensor("bq", [CD], F32, kind="ExternalInput").ap()
    bk = nc.dram_tensor("bk", [CD], F32, kind="ExternalInput").ap()
    out = nc.dram_tensor("out", [B, D, S], BF16, kind="ExternalOutput").ap()

    with tile.TileContext(nc) as tc:
        _build_tile(nc, tc, xT, wqT, wkT, wvT, woT, bq, bk, out)

    nc.compile()
    return nc


def _build_tile(nc, tc, xT, wqT, wkT, wvT, woT, bq, bk, out):
    import contextlib

    ctx = contextlib.ExitStack()
    with ctx:
        const = ctx.enter_context(tc.tile_pool(name="const", bufs=1))
        xpool = ctx.enter_context(tc.tile_pool(name="x", bufs=3))
        qkv = ctx.enter_context(tc.tile_pool(name="qkv", bufs=2))
        mt_p = ctx.enter_context(tc.tile_pool(name="mt", bufs=4))
        est_p = ctx.enter_context(tc.tile_pool(name="est", bufs=10))
        small = ctx.enter_context(tc.tile_pool(name="small", bufs=4))
        outp = ctx.enter_context(tc.tile_pool(name="outp", bufs=12))
        # PSUM budget (8 banks): ps_mm 2x2 + ps_acc 2x1 + ps_o 2x1 = 8
        ps_mm = ctx.enter_context(tc.tile_pool(name="ps_mm", bufs=2, space="PSUM"))
        ps_acc = ctx.enter_context(tc.tile_pool(name="ps_acc", bufs=2, space="PSUM"))
        ps_o = ctx.enter_context(tc.tile_pool(name="ps_o", bufs=2, space="PSUM"))

        xTr = xT.rearrange("(c p) t -> p c t", p=128)
        wqTr = wqT.rearrange("(c p) m -> p c m", p=128)
        wkTr = wkT.rearrange("(c p) m -> p c m", p=128)
        wvTr = wvT.rearrange("(c p) m -> p c m", p=128)

        wq_sb = const.tile([128, KC, CD], BF16, tag="wq")
        wk_sb = const.tile([128, KC, CD], BF16, tag="wk")
        wv_sb = const.tile([128, KC, CD], BF16, tag="wv")
        wo_sb = const.tile([128, HPC, D], BF16, tag="wo")
        bq_sb = const.tile([128, HPC], F32, tag="bq")
        bk_sb = const.tile([128, HPC], F32, tag="bk")
        ones_sb = const.tile([128, 128], BF16, tag="ones")

        xt0 = xpool.tile([128, KC, TT], BF16, tag="xt")

        # ---- staging: ci-interleaved just-in-time order for the cold tile.
        # x chunks ride the Act HWDGE queue (idle during P1) so weight and x
        # issue overlap; weights stay on SP.
        nc.sync.dma_start(xt0[:, 0:1, :], xTr[:, 0:1, 0:TT])
        nc.sync.dma_start(wq_sb[:, 0:1, :], wqTr[:, 0:1, :])
        nc.sync.dma_start(wk_sb[:, 0:1, :], wkTr[:, 0:1, :])
        nc.sync.dma_start(wv_sb[:, 0:1, :], wvTr[:, 0:1, :])
        for lo, hi in ((1, 4), (4, 8), (8, 12), (12, 16)):
            nc.sync.dma_start(xt0[:, lo:hi, :], xTr[:, lo:hi, 0:TT])
            nc.sync.dma_start(wq_sb[:, lo:hi, :], wqTr[:, lo:hi, :])
            nc.sync.dma_start(wk_sb[:, lo:hi, :], wkTr[:, lo:hi, :])
            nc.sync.dma_start(wv_sb[:, lo:hi, :], wvTr[:, lo:hi, :])
            if lo == 1:
                nc.sync.dma_start(bq_sb[:], bq.rearrange("(h p) -> p h", p=128))
                nc.sync.dma_start(bk_sb[:], bk.rearrange("(h p) -> p h", p=128))
        nc.vector.memset(ones_sb[:], 1.0)

        filler = []  # deferred out-projection emitters

        def pump(n=1):
            for _ in range(n):
                while filler:
                    if next(filler[0], None) is None:
                        filler.pop(0)
                    else:
                        break

        states = {}

        def get_state(b):
            if b not in states:
                states[b] = {
                    "QT": qkv.tile([128, HPC, S], BF16, tag="QT", name=f"QT{b}"),
                    "KT": qkv.tile([128, HPC, S], BF16, tag="KT", name=f"KT{b}"),
                    "V": qkv.tile([128, NB, CD], BF16, tag="V", name=f"V{b}"),
                }
            return states[b]

        def p1_tile_cold():
            """b=0,t=0: all 8 chains ci-interleaved so PE streams at DMA rate."""
            st = get_state(0)
            QT, KT, V = st["QT"], st["KT"], st["V"]
            qkA = ps_mm.tile([128, 2, TT], F32, tag="mm")  # h0: [Q | K]
            qkB = ps_mm.tile([128, 2, TT], F32, tag="mm")  # h1
            v0 = ps_acc.tile([128, CD], F32, tag="acc")  # one bank per open chain
            v1 = ps_acc.tile([128, CD], F32, tag="acc")
            # phase A: only Q chains for ci 0-3 (only xt+wq have landed)
            for ci in range(4):
                nc.tensor.matmul(qkA[:, 0, :], wq_sb[:, ci, 0:HD], xt0[:, ci, :], start=(ci == 0), stop=False)
                nc.tensor.matmul(qkB[:, 0, :], wq_sb[:, ci, HD:CD], xt0[:, ci, :], start=(ci == 0), stop=False)
            # phase B: K/V catch up on ci 0-3 while later chunks stream in
            for ci in range(4):
                nc.tensor.matmul(qkA[:, 1, :], wk_sb[:, ci, 0:HD], xt0[:, ci, :], start=(ci == 0), stop=False)
                nc.tensor.matmul(qkB[:, 1, :], wk_sb[:, ci, HD:CD], xt0[:, ci, :], start=(ci == 0), stop=False)
                nc.tensor.matmul(v0[:], xt0[:, ci, 0:128], wv_sb[:, ci, :], start=(ci == 0), stop=False)
                nc.tensor.matmul(v1[:], xt0[:, ci, 128:256], wv_sb[:, ci, :], start=(ci == 0), stop=False)
            # phase C: everything for ci 4-15
            for ci in range(4, KC):
                so = ci == KC - 1
                nc.tensor.matmul(qkA[:, 0, :], wq_sb[:, ci, 0:HD], xt0[:, ci, :], start=False, stop=so)
                nc.tensor.matmul(qkA[:, 1, :], wk_sb[:, ci, 0:HD], xt0[:, ci, :], start=False, stop=so)
                nc.tensor.matmul(qkB[:, 0, :], wq_sb[:, ci, HD:CD], xt0[:, ci, :], start=False, stop=so)
                nc.tensor.matmul(qkB[:, 1, :], wk_sb[:, ci, HD:CD], xt0[:, ci, :], start=False, stop=so)
                nc.tensor.matmul(v0[:], xt0[:, ci, 0:128], wv_sb[:, ci, :], start=False, stop=so)
                nc.tensor.matmul(v1[:], xt0[:, ci, 128:256], wv_sb[:, ci, :], start=False, stop=so)
            # drain h0 first so the next tile's first chain gets its PSUM slot
            # back; the tb2/tb3 V pass below covers all four Act drains
            nc.scalar.activation(QT[:, 0, 0:TT], qkA[:, 0, :], Act.Identity, bias=bq_sb[:, 0:1])
            nc.scalar.activation(KT[:, 0, 0:TT], qkA[:, 1, :], Act.Identity, bias=bk_sb[:, 0:1])
            nc.vector.tensor_copy(V[:, 0, :], v0[:])
            nc.vector.tensor_copy(V[:, 1, :], v1[:])
            v2 = ps_acc.tile([128, CD], F32, tag="acc")
            v3 = ps_acc.tile([128, CD], F32, tag="acc")
            for ci in range(KC):
                sa = ci == 0
                so = ci == KC - 1
                nc.tensor.matmul(v2[:], xt0[:, ci, 256:384], wv_sb[:, ci, :], start=sa, stop=so)
                nc.tensor.matmul(v3[:], xt0[:, ci, 384:512], wv_sb[:, ci, :], start=sa, stop=so)
                if ci == 1:
                    nc.scalar.activation(QT[:, 1, 0:TT], qkB[:, 0, :], Act.Identity, bias=bq_sb[:, 1:2])
                    nc.scalar.activation(KT[:, 1, 0:TT], qkB[:, 1, :], Act.Identity, bias=bk_sb[:, 1:2])
            nc.vector.tensor_copy(V[:, 2, :], v2[:])
            nc.vector.tensor_copy(V[:, 3, :], v3[:])

        xts = {}

        def p1_tile(b, t, boundary=False, skip_v=False):
            st = get_state(b)
            QT, KT, V = st["QT"], st["KT"], st["V"]
            off = b * S + t * TT
            if (b, t) in xts:
                xt = xts[(b, t)]
            else:
                xt = xpool.tile([128, KC, TT], BF16, tag="xt")
                xts[(b, t)] = xt
                for lo, hi in ((0, 8), (8, 16)):
                    nc.sync.dma_start(xt[:, lo:hi, :], xTr[:, lo:hi, off : off + TT])
            for h in range(HPC):
                if boundary and h == 0:
                    # ps_o is idle during P1: using it here means the first
                    # chains after the cold tile don't wait on its Act drains
                    q_ps = ps_o.tile([128, TT], F32, tag="o")
                    k_ps = ps_o.tile([128, TT], F32, tag="o")
                else:
                    qk = ps_mm.tile([128, 2, TT], F32, tag="mm")
                    q_ps = qk[:, 0, :]
                    k_ps = qk[:, 1, :]
                mo = h * HD
                for ci in range(KC):
                    nc.tensor.matmul(
                        q_ps[:], wq_sb[:, ci, mo : mo + HD], xt[:, ci, :],
                        start=(ci == 0), stop=(ci == KC - 1),
                    )
                nc.scalar.activation(
                    QT[:, h, t * TT : (t + 1) * TT], q_ps[:], Act.Identity,
                    bias=bq_sb[:, h : h + 1],
                )
                pump(1)
                for ci in range(KC):
                    nc.tensor.matmul(
                        k_ps[:], wk_sb[:, ci, mo : mo + HD], xt[:, ci, :],
                        start=(ci == 0), stop=(ci == KC - 1),
                    )
                nc.scalar.activation(
                    KT[:, h, t * TT : (t + 1) * TT], k_ps[:], Act.Identity,
                    bias=bk_sb[:, h : h + 1],
                )
            if skip_v:
                return
            for tb in range(4):
                v_ps = ps_acc.tile([128, CD], F32, tag="acc")
                for ci in range(KC):
                    nc.tensor.matmul(
                        v_ps[:], xt[:, ci, tb * 128 : (tb + 1) * 128],
                        wv_sb[:, ci, :], start=(ci == 0), stop=(ci == KC - 1),
                    )
                nc.vector.tensor_copy(V[:, t * 4 + tb, :], v_ps[:])

        def v_defer_items(b, t):
            """One P1 tile's V chains as 16 slot-sized pieces (4 accumulating
            matmuls each) to fill qt0's score slots; PSUM from the otherwise
            idle ps_o pool, one bank per open chain (zero-region rule)."""
            st = get_state(b)
            V = st["V"]
            xt = xts[(b, t)]
            tiles = {}
            items = []
            for pair in (0, 1):
                for q in range(4):
                    for tb in (2 * pair, 2 * pair + 1):
                        def emit(q=q, tb=tb):
                            if tb not in tiles:
                                tiles[tb] = ps_o.tile(
                                    [128, CD], F32, tag="o", name=f"vd{b}_{t}_{tb}"
                                )
                            v_ps = tiles[tb]
                            for ci in range(4 * q, 4 * q + 4):
                                nc.tensor.matmul(
                                    v_ps[:], xt[:, ci, tb * 128 : (tb + 1) * 128],
                                    wv_sb[:, ci, :], start=(ci == 0), stop=(ci == KC - 1),
                                )
                            if q == 3:
                                nc.vector.tensor_copy(V[:, 4 * t + tb, :], v_ps[:])
                        items.append(emit)
            return items

        def attention(b, qt, v_items=None, v1_items=None, qbase=None, W=TT):
            st = get_state(b)
            QT, KT, V = st["QT"], st["KT"], st["V"]
            if qbase is None:
                qbase = qt * TT
            qsl = slice(qbase, qbase + W)
            MT = mt_p.tile([128, HPC, TT], BF16, tag="MT")
            for h in range(HPC):
                attn_ps = ps_acc.tile([128, TT], F32, tag="acc")
                dacc = small.tile([128, 2, TT], BF16, tag="dacc")
                dsum = small.tile([128, TT], BF16, tag="dsum")
                ests = []
                for kp in range(NB // 2):
                    st_ps = ps_mm.tile([128, 2, TT], F32, tag="mm")
                    for j in (0, 1):
                        kb = 2 * kp + j
                        nc.tensor.matmul(
                            st_ps[:, j, 0:W], KT[:, h, kb * 128 : (kb + 1) * 128],
                            QT[:, h, qsl], start=True, stop=True,
                        )
                    est = est_p.tile([128, 2, TT], BF16, tag="est")
                    nc.scalar.activation(est[:, :, 0:W], st_ps[:, :, 0:W], Act.Exp, scale=SCALE)
                    ests.append(est)
                    if kp == 0:
                        nc.vector.tensor_copy(dacc[:, :, 0:W], est[:, :, 0:W])
                    else:
                        nc.vector.tensor_add(dacc[:, :, 0:W], dacc[:, :, 0:W], est[:, :, 0:W])
                    if h == 0 and v_items:
                        v_items.pop(0)()
                        if len(v_items) > 8 - kp:
                            v_items.pop(0)()
                    elif h == 1 and v1_items:
                        v1_items.pop(0)()
                        if len(v1_items) > 8 - kp:
                            v1_items.pop(0)()
                    elif h == 1 and v_items is not None:
                        # qt0: two P3 blocks per h1 slot clears the previous
                        # batch's last MT without touching P1
                        pump(2)
                    elif kp < 6:
                        pump(1)
                if h == 0 and v_items:
                    while v_items:
                        v_items.pop(0)()
                elif h == 1 and v1_items:
                    while v1_items:
                        v1_items.pop(0)()
                nc.vector.tensor_add(dsum[:, 0:W], dacc[:, 0, 0:W], dacc[:, 1, 0:W])
                # ps_acc, not ps_o: sharing the P3 o_ps rotation makes early
                # P3 blocks of the next qt wait on this tile's reciprocal
                dn_ps = ps_acc.tile([128, TT], F32, tag="acc")
                recip = small.tile([128, TT], F32, tag="recip")
                for kp in range(NB // 2):
                    for j in (0, 1):
                        kb = 2 * kp + j
                        nc.tensor.matmul(
                            attn_ps[:, 0:W], V[:, kb, h * HD : (h + 1) * HD],
                            ests[kp][:, j, 0:W], start=(kb == 0), stop=(kb == NB - 1),
                        )
                    if kp == 4:
                        nc.tensor.matmul(dn_ps[:, 0:W], ones_sb[:], dsum[:, 0:W], start=True, stop=True)
                        nc.vector.reciprocal(recip[:, 0:W], dn_ps[:, 0:W])
                    elif kp < 2 and not (v_items is not None or v1_items):
                        pump(1)
                nc.vector.tensor_mul(MT[:, h, 0:W], attn_ps[:, 0:W], recip[:, 0:W])
            return MT

        def p3_steps(MT, b, qsl, drain=False, tail_act=False):
            """Generator: one out-projection step per next().

            Steady state: one 128-row block per step, PSUM from ps_o, copy on
            DVE (3 of 4) / Act (1 of 4), store on the gpsimd SWDGE queue so
            loads never head-of-line block.
            Drain (after the last attention): two blocks per step in a
            [128,2,TT] tile from the freed ps_mm pool — one wide copy and one
            256KB store per step so copies/stores keep pace with the 852ns
            matmul cadence; stores round-robin all three DMA queues."""
            if drain:
                # h0 matmuls for the first six blocks run before MT[:,1,:]
                # lands (it trails behind the recip -> MT-mul DVE chain):
                # 2 ps_o singles + 2 ps_mm pairs = 6 open accumulations
                o_f = [ps_o.tile([128, TT], F32, tag="o", name=f"of{b}_{d}") for d in (0, 1)]
                o_p = [
                    ps_mm.tile([128, 2, TT], F32, tag="mm", name=f"op{b}_{p}")
                    for p in (1, 2)
                ]

                def mm_into(d, h, stop):
                    tgt = o_f[d][:] if d < 2 else o_p[(d - 2) // 2][:, d % 2, :]
                    nc.tensor.matmul(
                        tgt, wo_sb[:, h, d * 128 : (d + 1) * 128],
                        MT[:, h, :], start=(h == 0), stop=stop,
                    )

                for d in range(6):
                    mm_into(d, 0, stop=False)
                for d in range(6):
                    mm_into(d, 1, stop=True)
                for d in (0, 1):
                    o_sb = outp.tile([128, TT], BF16, tag="o", name=f"o_sbf{b}_{d}")
                    if d == 0:
                        nc.vector.tensor_copy(o_sb[:], o_f[d][:])
                    else:
                        nc.scalar.copy(o_sb[:], o_f[d][:])
                    nc.gpsimd.dma_start(out[b, d * 128 : (d + 1) * 128, qsl], o_sb[:])
                    yield d
                engs = itertools.cycle([nc.sync, nc.gpsimd, nc.scalar, nc.sync, nc.scalar])
                outR = out.rearrange("b (k j p) t -> b k p j t", j=2, p=128)
                for pair in (1, 2):
                    o_sb = outp.tile(
                        [128, 2, TT], BF16, tag="od", bufs=6, name=f"o_sbd{b}_{pair}"
                    )
                    if pair % 2 == 1:
                        nc.vector.tensor_copy(o_sb[:], o_p[pair - 1][:])
                    else:
                        nc.scalar.copy(o_sb[:], o_p[pair - 1][:])
                    next(engs).dma_start(outR[b, pair, :, :, qsl], o_sb[:])
                    yield pair
                for pair in range(3, D // 256 - 1):
                    o_ps = ps_mm.tile([128, 2, TT], F32, tag="mm", name=f"op{b}_{pair}")
                    for j in (0, 1):
                        for h in range(HPC):
                            nc.tensor.matmul(
                                o_ps[:, j, :],
                                wo_sb[:, h, (2 * pair + j) * 128 : (2 * pair + j + 1) * 128],
                                MT[:, h, :], start=(h == 0), stop=(h == HPC - 1),
                            )
                    o_sb = outp.tile(
                        [128, 2, TT], BF16, tag="od", bufs=6, name=f"o_sbd{b}_{pair}"
                    )
                    if pair % 2 == 0:
                        nc.vector.tensor_copy(o_sb[:], o_ps[:])
                    else:
                        nc.scalar.copy(o_sb[:], o_ps[:])
                    next(engs).dma_start(outR[b, pair, :, :, qsl], o_sb[:])
                    yield pair
                # last two blocks split fine so the final copies and stores
                # ride short-latency parallel paths
                for dblk in (14, 15):
                    o_ps = ps_o.tile([128, TT], F32, tag="o", name=f"ol{b}_{dblk}")
                    for h in range(HPC):
                        nc.tensor.matmul(
                            o_ps[:], wo_sb[:, h, dblk * 128 : (dblk + 1) * 128],
                            MT[:, h, :], start=(h == 0), stop=(h == HPC - 1),
                        )
                    o_sb = outp.tile([128, TT], BF16, tag="o", name=f"o_sbl{b}_{dblk}")
                    if dblk == 14:
                        nc.vector.tensor_copy(o_sb[:], o_ps[:])
                        nc.sync.dma_start(
                            out[b, dblk * 128 : (dblk + 1) * 128, qsl], o_sb[:]
                        )
                    else:
                        nc.vector.tensor_copy(o_sb[:, 0 : TT // 2], o_ps[:, 0 : TT // 2])
                        nc.scalar.copy(o_sb[:, TT // 2 : TT], o_ps[:, TT // 2 : TT])
                        qlo = qsl.start
                        nc.sync.dma_start(
                            out[b, dblk * 128 : (dblk + 1) * 128, qlo : qlo + TT // 2],
                            o_sb[:, 0 : TT // 2],
                        )
                        nc.scalar.dma_start(
                            out[b, dblk * 128 : (dblk + 1) * 128, qlo + TT // 2 : qlo + TT],
                            o_sb[:, TT // 2 : TT],
                        )
                    yield dblk
                return
            for dblk in range(D // 128):
                o_ps = ps_o.tile([128, TT], F32, tag="o", name=f"o_ps{b}_{dblk}")
                for h in range(HPC):
                    nc.tensor.matmul(
                        o_ps[:], wo_sb[:, h, dblk * 128 : (dblk + 1) * 128],
                        MT[:, h, :], start=(h == 0), stop=(h == HPC - 1),
                    )
                o_sb = outp.tile([128, TT], BF16, tag="o", name=f"o_sb{b}_{dblk}")
                # Act must never carry too many P3 copies (head-of-line
                # ahead of the exp stream); before a drain, keep DVE clear
                # for the recip -> MT chain instead
                if dblk % 4 == 3 or (tail_act and dblk >= 12):
                    nc.scalar.copy(o_sb[:], o_ps[:])
                else:
                    nc.vector.tensor_copy(o_sb[:], o_ps[:])
                nc.gpsimd.dma_start(out[b, dblk * 128 : (dblk + 1) * 128, qsl], o_sb[:])
                yield dblk

        # ---- batch 0 ----
        p1_tile_cold()
        p1_tile(0, 1, boundary=True)
        p1_tile(0, 2)
        p1_tile(0, 3, skip_v=True)
        # wo is first needed by P3 of (b0,qt0), injected during (b0,qt1)
        woTr = woT.rearrange("(h p) m -> p h m", p=128)
        nc.sync.dma_start(wo_sb[:, 0:1, :], woTr[:, 0:1, :])
        nc.sync.dma_start(wo_sb[:, 1:2, :], woTr[:, 1:2, :])
        # prefetch the next batch's first x tile; its V chains fill the
        # otherwise starved (b0,qt0) h1 score slots
        xt10 = xpool.tile([128, KC, TT], BF16, tag="xt", name="xt10")
        xts[(1, 0)] = xt10
        for lo, hi in ((0, 8), (8, 16)):
            nc.sync.dma_start(xt10[:, lo:hi, :], xTr[:, lo:hi, S : S + TT])
        for qt in range(NQ):
            v_items = v_defer_items(0, 3) if qt == 0 else None
            v1_items = v_defer_items(1, 0) if qt == 0 else None
            MT = attention(0, qt, v_items, v1_items)
            filler.append(p3_steps(MT, 0, slice(qt * TT, (qt + 1) * TT)))
        # ---- batch 1 ----
        p1_tile(1, 0, skip_v=True)
        p1_tile(1, 1)
        p1_tile(1, 2)
        p1_tile(1, 3, skip_v=True)
        states.pop(0)
        for qt in range(NQ):
            v_items = v_defer_items(1, 3) if qt == 0 else None
            MT = attention(1, qt, v_items)
            filler.append(
                p3_steps(
                    MT, 1, slice(qt * TT, (qt + 1) * TT),
                    drain=(qt == NQ - 1),
                )
            )
        while filler:
            if next(filler[0], None) is None:
                filler.pop(0)


_program = None


def _get_program():
    global _program
    if _program is None:
        _program = build_program()
    return _program


def kernel(x, Wq, bq, Wk, bk, Wv, bv, Wo, bo):
    x = np.asarray(x, np.float32)
    Wq, Wk, Wv, Wo = (np.asarray(w, np.float32) for w in (Wq, Wk, Wv, Wo))
    bq, bk, bv, bo = (np.asarray(v, np.float32) for v in (bq, bk, bv, bo))

    bf = ml_dtypes.bfloat16
    xT = np.ascontiguousarray(x.reshape(TOK, D).T).astype(bf)

    nc = _get_program()
    in_maps = []
    for c in range(N_CORES):
        sl = slice(c * CD, (c + 1) * CD)
        in_maps.append(
            {
                "xT": xT,
                "wqT": np.ascontiguousarray(Wq[sl, :].T).astype(bf),
                "wkT": np.ascontiguousarray(Wk[sl, :].T).astype(bf),
                "wvT": np.ascontiguousarray(Wv[sl, :].T).astype(bf),
                "woT": np.ascontiguousarray(Wo[:, sl].T).astype(bf),
                "bq": np.ascontiguousarray(bq[sl]),
                "bk": np.ascontiguousarray(bk[sl]),
            }
        )

    res = run_bass_kernel_spmd(nc, in_maps, core_ids=list(range(N_CORES)))
    acc = np.zeros((B, D, S), np.float32)
    for r in res.results:
        acc += np.asarray(r["out"], np.float32)
    return np.ascontiguousarray(acc.transpose(0, 2, 1)) + (bo + Wo @ bv)
